# revision 1
# baseline (speedup 1.0000x reference)
"""Trainium2 Bass kernel for nn_CNN_LeNet_83794811945244 (AdderNet LeNet).

Mathematical structure
----------------------
``adder2d`` computes ``-sum |x_patch - w|`` which is **<= 0 for every input**
(it is a negated sum of absolute values).  The reference net applies
``relu`` directly to each adder output, so both adder stages are
identically zero for ANY input tensors of the given shapes:

  * layer1: ``relu(adder2d(x, w1)) == 0``; training-mode batchnorm of an
    all-zero tensor is exactly ``beta1`` ((0-mean)=0 exactly, so the
    rsqrt(var+eps) factor multiplies 0); maxpool of a constant is the
    constant.
  * layer2 input is the constant image ``beta1``; again
    ``relu(adder2d(.)) == 0``; bn -> ``beta2``; pool -> ``beta2``.
  * flattened features: ``h[f] = beta2[f // 25]`` (f = (o, 5, 5)).

Therefore every output row equals
``softmax(fc3(relu(fc2(relu(fc1(h))))))`` - batch independent.  The kernel
computes that (input-dependent!) row on each NeuronCore from the real
``bn2_beta`` / fc weights and broadcasts it over its batch shard.

Sharding: pure data parallel over the batch (1024 -> 8 x 128); weights
replicated (the hint's strategy) - each core produces its own [128, 10]
shard, which the host concatenates.
"""
import sys
import numpy as np

for _p in ("/opt/trn_rl_repo",):
    if _p not in sys.path:
        sys.path.insert(0, _p)

import concourse.bass as bass  # noqa: E402
import concourse.tile as tile  # noqa: E402
from concourse import bacc, mybir  # noqa: E402
from concourse.bass_utils import run_bass_kernel_spmd  # noqa: E402
from contextlib import ExitStack  # noqa: E402

F32 = mybir.dt.float32
OP = mybir.AluOpType
AF = mybir.ActivationFunctionType
AX = mybir.AxisListType

NCORES = 8
BSHARD = 128  # batch rows per core


def _host_constants():
    exp400 = np.zeros((16, 400), dtype=np.float32)
    for f in range(400):
        exp400[f // 25, f] = 1.0
    return {
        "c_exp400": exp400,
        "c_ones128": np.ones((1, 128), dtype=np.float32),
        "c_id128f": np.eye(128, dtype=np.float32),
    }


def _build(nc, tc, ctx):
    pool = ctx.enter_context(tc.tile_pool(name="p", bufs=1))
    psum = ctx.enter_context(tc.tile_pool(name="ps", bufs=1, space="PSUM"))

    dp = lambda n, s: nc.declare_dram_parameter(n, list(s), F32, isOutput=False)
    b2_d = dp("bn2_beta_col", [16, 1])
    w1t_d = dp("fc1_w_t", [400, 120])
    b1_d = dp("fc1_b_col", [120, 1])
    w2t_d = dp("fc2_w_t", [120, 84])
    b2c_d = dp("fc2_b_col", [84, 1])
    w3t_d = dp("fc3_w_t", [84, 10])
    b3_d = dp("fc3_b_col", [10, 1])
    exp_d = dp("c_exp400", [16, 400])
    id_d = dp("c_id128f", [128, 128])
    ones_d = dp("c_ones128", [1, 128])
    out_d = nc.declare_dram_parameter("out", [BSHARD, 10], F32, isOutput=True)

    czero = pool.tile([128, 1], F32)
    nc.gpsimd.memset(czero[:], 0.0)
    nc.const_aps.aps[(F32, 0.0)] = czero[:]

    _ctr = [0]

    def t(s):
        _ctr[0] += 1
        return pool.tile(list(s), F32, name=f"t{_ctr[0]}")

    beta2 = t([16, 1]); nc.sync.dma_start(beta2[:], b2_d[:])
    exp4 = t([16, 400]); nc.sync.dma_start(exp4[:], exp_d[:])
    w1ts = []
    for k in range(4):
        w = pool.tile([100, 120], F32, name=f"w1t{k}")
        nc.sync.dma_start(w[:], w1t_d[k * 100:(k + 1) * 100, :])
        w1ts.append(w)
    fc1b = t([120, 1]); nc.sync.dma_start(fc1b[:], b1_d[:])
    w2t = t([120, 84]); nc.sync.dma_start(w2t[:], w2t_d[:])
    fc2b = t([84, 1]); nc.sync.dma_start(fc2b[:], b2c_d[:])
    w3t = t([84, 10]); nc.sync.dma_start(w3t[:], w3t_d[:])
    fc3b = t([10, 1]); nc.sync.dma_start(fc3b[:], b3_d[:])
    ones128 = t([1, 128]); nc.sync.dma_start(ones128[:], ones_d[:])

    # h chunks [100,1] = exp_chunk.T @ beta2   (h[f] = beta2[f//25])
    hcol = t([100, 4])
    for k in range(4):
        ps = psum.tile([100, 1], F32, name=f"hps{k}", tag="hps")
        nc.tensor.matmul(ps[:], exp4[:, k * 100:(k + 1) * 100], beta2[:],
                         start=True, stop=True)
        nc.scalar.activation(hcol[:, k:k + 1], ps[:], AF.Copy)

    # FC1 (120) -> relu
    f1ps = psum.tile([120, 1], F32, name="f1ps")
    for k in range(4):
        nc.tensor.matmul(f1ps[:], w1ts[k][:], hcol[:, k:k + 1],
                         start=(k == 0), stop=(k == 3))
    f1 = t([120, 1])
    nc.scalar.activation(f1[:], f1ps[:], AF.Relu, bias=fc1b[:])

    # FC2 (84) -> relu
    f2ps = psum.tile([84, 1], F32, name="f2ps")
    nc.tensor.matmul(f2ps[:], w2t[:], f1[:], start=True, stop=True)
    f2 = t([84, 1])
    nc.scalar.activation(f2[:], f2ps[:], AF.Relu, bias=fc2b[:])

    # FC3 (10) + bias
    f3ps = psum.tile([10, 1], F32, name="f3ps")
    nc.tensor.matmul(f3ps[:], w3t[:], f2[:], start=True, stop=True)
    z = t([10, 1])
    nc.scalar.activation(z[:], f3ps[:], AF.Identity, bias=fc3b[:])

    # transpose z -> row [1, 10]
    id128 = t([128, 128]); nc.sync.dma_start(id128[:], id_d[:])
    zps = psum.tile([1, 16], F32, name="zps")
    nc.tensor.transpose(zps[:, 0:10], z[:], id128[0:10, 0:10])
    zrow = t([1, 16])
    nc.scalar.activation(zrow[:, 0:10], zps[:, 0:10], AF.Copy)

    # softmax on the row (fp32)
    zmax = t([1, 1])
    nc.vector.tensor_reduce(zmax[:], zrow[:, 0:10], AX.X, OP.max)
    ze = t([1, 10])
    nc.vector.tensor_scalar(ze[:], zrow[:, 0:10], zmax[:], None, op0=OP.subtract)
    nc.scalar.activation(ze[:], ze[:], AF.Exp)
    zsum = t([1, 1])
    nc.vector.tensor_reduce(zsum[:], ze[:], AX.X, OP.add)
    zr = t([1, 1])
    nc.vector.reciprocal(zr[:], zsum[:])
    prob = t([1, 10])
    nc.vector.tensor_scalar(prob[:], ze[:], zr[:], None, op0=OP.mult)

    # broadcast row across the 128 batch partitions and store
    ops = psum.tile([BSHARD, 10], F32, name="ops")
    nc.tensor.matmul(ops[:], ones128[:], prob[:], start=True, stop=True)
    osb = t([BSHARD, 10])
    nc.scalar.activation(osb[:], ops[:], AF.Copy)
    nc.sync.dma_start(out_d[:], osb[:])


_COMPILED = None


def _get_compiled():
    global _COMPILED
    if _COMPILED is None:
        nc = bacc.Bacc()
        with tile.TileContext(nc) as tc:
            with ExitStack() as ctx:
                _build(nc, tc, ctx)
        nc.compile()
        _COMPILED = nc
    return _COMPILED


def kernel(**inputs) -> np.ndarray:
    nc = _get_compiled()
    m = {
        "bn2_beta_col": np.asarray(inputs["bn2_beta"], np.float32).reshape(16, 1),
        "fc1_w_t": np.ascontiguousarray(np.asarray(inputs["fc1_w"], np.float32).T),
        "fc1_b_col": np.asarray(inputs["fc1_b"], np.float32).reshape(120, 1),
        "fc2_w_t": np.ascontiguousarray(np.asarray(inputs["fc2_w"], np.float32).T),
        "fc2_b_col": np.asarray(inputs["fc2_b"], np.float32).reshape(84, 1),
        "fc3_w_t": np.ascontiguousarray(np.asarray(inputs["fc3_w"], np.float32).T),
        "fc3_b_col": np.asarray(inputs["fc3_b"], np.float32).reshape(10, 1),
    }
    m.update(_host_constants())
    res = run_bass_kernel_spmd(nc, [dict(m) for _ in range(NCORES)],
                               list(range(NCORES)))
    out = np.concatenate([res.results[c]["out"] for c in range(NCORES)], axis=0)
    batch = int(np.asarray(inputs["x"]).shape[0])
    return out[:batch].astype(np.float32)


# revision 2
# speedup vs baseline: 1.3176x; 1.3176x over previous
"""Trainium2 Bass kernel for nn_CNN_LeNet_83794811945244 (AdderNet LeNet).

Mathematical structure
----------------------
``adder2d`` returns ``-sum |x_patch - w|``, which is **<= 0 for every
possible input** (a negated sum of absolute values).  The reference net
applies ``relu`` directly to each adder output, so both adder stages are
identically zero for ANY input tensors of these shapes:

  * layer1: ``relu(adder2d(x, w1)) == 0`` elementwise; training-mode
    batchnorm of the all-zero tensor is exactly ``beta1`` (the ``0 - mean``
    numerator is exactly 0, so the ``rsqrt(var + eps)`` factor multiplies
    0); maxpool of a constant is that constant.
  * layer2 sees the constant image ``beta1``; again
    ``relu(adder2d(.)) == 0``; bn -> ``beta2``; pool -> ``beta2``.
  * flattened features: ``h[f] = beta2[f // 25]``  (f = (channel, 5, 5)).

Every output row therefore equals
``softmax(fc3_b + fc3_w @ relu(fc2_b + fc2_w @ relu(fc1_b + fc1_w @ h)))``
- input-data independent but *weight*-dependent.  The kernel computes that
row on each NeuronCore from the real ``bn2_beta`` / fc weights (exact fp32
constant-folding of the network; no approximation) and broadcasts it over
its batch shard.

Sharding: pure data parallel over batch (1024 -> 8 x 128) per the hint;
weights replicated.  Each core produces its own [128, 10] shard; the host
concatenates.

Device-side pipeline (all fp32, exact):
  one packed-weights DMA ->
  G.T[16,120] = sum_k expT_k.T @ fc1_w.T_k   (PE, folds h-expansion)
  f1 = relu(G.T.T @ beta2 + b1)              (PE + DVE add/max)
  f2 = relu(fc2_w.T.T @ f1 + b2)             (PE + DVE)
  z  = f2.T @ fc3_w.T + b3                   (PE row-form, no transpose)
  softmax row (DVE max/sum/reciprocal + ACT exp, table preloaded)
  broadcast store via step-0 DMA replication.
"""
import sys
import numpy as np

for _p in ("/opt/trn_rl_repo",):
    if _p not in sys.path:
        sys.path.insert(0, _p)

import concourse.bass as bass  # noqa: E402
import concourse.tile as tile  # noqa: E402
from concourse import bacc, mybir  # noqa: E402
from concourse.bass_utils import run_bass_kernel_spmd  # noqa: E402
from contextlib import ExitStack  # noqa: E402

F32 = mybir.dt.float32
OP = mybir.AluOpType
AF = mybir.ActivationFunctionType
AX = mybir.AxisListType

NCORES = 8
BSHARD = 128

# packed [128, PCOLS] fp32 layout:
#  0:480    fc1_w.T chunks k=0..3 at [0:100, 120k:120k+120]
#  480:564  fc2_w.T [0:120]
#  564:574  fc3_w.T [0:84]
#  574      fc1_b col [0:120]
#  575      fc2_b col [0:84]
#  576      bn2_beta col [0:16]
#  577:641  expT chunks k=0..3 at [0:100, 577+16k:577+16k+16]
#  641:651  fc3_b row [0:1]
#  651      one  [0:1]
PCOLS = 652


def _pack_inputs(inputs):
    P = np.zeros((128, PCOLS), dtype=np.float32)
    w1t = np.asarray(inputs["fc1_w"], np.float32).T  # [400, 120]
    for k in range(4):
        P[0:100, 120 * k:120 * k + 120] = w1t[100 * k:100 * k + 100]
    P[0:120, 480:564] = np.asarray(inputs["fc2_w"], np.float32).T
    P[0:84, 564:574] = np.asarray(inputs["fc3_w"], np.float32).T
    P[0:120, 574] = np.asarray(inputs["fc1_b"], np.float32).ravel()
    P[0:84, 575] = np.asarray(inputs["fc2_b"], np.float32).ravel()
    P[0:16, 576] = np.asarray(inputs["bn2_beta"], np.float32).ravel()
    expT = np.zeros((400, 16), dtype=np.float32)
    for f in range(400):
        expT[f, f // 25] = 1.0
    for k in range(4):
        P[0:100, 577 + 16 * k:577 + 16 * k + 16] = expT[100 * k:100 * k + 100]
    P[0, 641:651] = np.asarray(inputs["fc3_b"], np.float32).ravel()
    P[0, 651] = 1.0
    return {"packed": P}


def _build(nc, tc, ctx):
    pool = ctx.enter_context(tc.tile_pool(name="p", bufs=1))
    psum = ctx.enter_context(tc.tile_pool(name="ps", bufs=1, space="PSUM"))

    pk_d = nc.declare_dram_parameter("packed", [128, PCOLS], F32, isOutput=False)
    out_d = nc.declare_dram_parameter("out", [BSHARD, 10], F32, isOutput=True)

    pk = pool.tile([128, PCOLS], F32)
    nc.scalar.dma_start(pk[:], pk_d[:])

    # exp-table preload, overlapped with the DMA wait
    warm = pool.tile([1, 1], F32)
    nc.gpsimd.memset(warm[:], 0.0)
    nc.const_aps.aps[(F32, 0.0)] = warm[:]
    nc.scalar.activation(warm[:], warm[:], AF.Exp)

    w1t = lambda k: pk[0:100, 120 * k:120 * k + 120]
    w2t = pk[0:120, 480:564]
    w3r = pk[0:84, 564:574]
    b1c = pk[0:120, 574:575]
    b2c = pk[0:84, 575:576]
    beta2 = pk[0:16, 576:577]
    expT = lambda k: pk[0:100, 577 + 16 * k:577 + 16 * k + 16]
    b3row = pk[0:1, 641:651]
    ones1 = pk[0:1, 651:652]

    # G.T [16, 120] = sum_k expT_k.T @ w1t_k  (== group-summed fc1_w.T)
    gps = psum.tile([16, 120], F32, name="gps")
    for k in range(4):
        nc.tensor.matmul(gps[:], expT(k), w1t(k), start=(k == 0), stop=(k == 3))
    gt = pool.tile([16, 120], F32)
    nc.vector.tensor_copy(gt[:], gps[:])

    # FC1 + relu (DVE: (psum + b) max 0)
    f1ps = psum.tile([120, 1], F32, name="f1ps")
    nc.tensor.matmul(f1ps[:], gt[:], beta2, start=True, stop=True)
    f1 = pool.tile([120, 1], F32)
    nc.vector.tensor_scalar(f1[:], f1ps[:], b1c, 0.0, OP.add, OP.max)

    # FC2 + relu
    f2ps = psum.tile([84, 1], F32, name="f2ps")
    nc.tensor.matmul(f2ps[:], w2t, f1[:], start=True, stop=True)
    f2 = pool.tile([84, 1], F32)
    nc.vector.tensor_scalar(f2[:], f2ps[:], b2c, 0.0, OP.add, OP.max)

    # FC3 in row form [1, 10]; bias accumulated via K=1 matmul
    zps = psum.tile([1, 10], F32, name="zps")
    nc.tensor.matmul(zps[:], f2[:], w3r, start=True, stop=False)
    nc.tensor.matmul(zps[:], ones1, b3row, start=False, stop=True)

    # softmax on the row
    negmax = pool.tile([1, 1], F32)
    nc.vector.tensor_reduce(negmax[:], zps[:], AX.X, OP.max, negate=True)
    ze = pool.tile([1, 10], F32)
    nc.scalar.activation(ze[:], zps[:], AF.Exp, bias=negmax[:])
    zsum = pool.tile([1, 1], F32)
    nc.vector.tensor_reduce(zsum[:], ze[:], AX.X, OP.add)
    zr = pool.tile([1, 1], F32)
    nc.vector.reciprocal(zr[:], zsum[:])
    prob = pool.tile([1, 10], F32)
    nc.vector.tensor_scalar(prob[:], ze[:], zr[:], None, op0=OP.mult)

    # broadcast-store: step-0 DMA replicates the row to all 128 batch rows
    nc.sync.dma_start(
        out_d[:],
        prob[0:1, :].rearrange("p (a q) -> p a q", a=1).to_broadcast((1, BSHARD, 10)))


_COMPILED = None


def _get_compiled():
    global _COMPILED
    if _COMPILED is None:
        nc = bacc.Bacc()
        with tile.TileContext(nc) as tc:
            with ExitStack() as ctx:
                _build(nc, tc, ctx)
        nc.compile()
        _COMPILED = nc
    return _COMPILED


def kernel(**inputs) -> np.ndarray:
    nc = _get_compiled()
    m = _pack_inputs(inputs)
    res = run_bass_kernel_spmd(nc, [dict(m) for _ in range(NCORES)],
                               list(range(NCORES)))
    out = np.concatenate([res.results[c]["out"] for c in range(NCORES)], axis=0)
    batch = int(np.asarray(inputs["x"]).shape[0])
    return out[:batch].astype(np.float32)


# revision 3
# speedup vs baseline: 1.3571x; 1.0299x over previous
"""Trainium2 Bass kernel for nn_CNN_LeNet_83794811945244 (AdderNet LeNet).

Mathematical structure
----------------------
``adder2d`` returns ``-sum |x_patch - w|``, which is **<= 0 for every
possible input** (a negated sum of absolute values).  The reference net
applies ``relu`` directly to each adder output, so both adder stages are
identically zero for ANY input tensors of these shapes:

  * layer1: ``relu(adder2d(x, w1)) == 0`` elementwise; training-mode
    batchnorm of the all-zero tensor is exactly ``beta1`` (the ``0 - mean``
    numerator is exactly 0, so the ``rsqrt(var + eps)`` factor multiplies
    0); maxpool of a constant is that constant.
  * layer2 sees the constant image ``beta1``; again
    ``relu(adder2d(.)) == 0``; bn -> ``beta2``; pool -> ``beta2``.
  * flattened features: ``h[f] = beta2[f // 25]``  (f = (channel, 5, 5)).

Every output row therefore equals
``softmax(fc3_b + fc3_w @ relu(fc2_b + fc2_w @ relu(fc1_b + fc1_w @ h)))``
- input-data independent but *weight*-dependent.  The kernel computes that
row on each NeuronCore from the real ``bn2_beta`` / fc weights (exact fp32
constant-folding of the network; no approximation) and broadcasts it over
its batch shard.

Sharding: pure data parallel over batch (1024 -> 8 x 128) per the hint;
weights replicated.  Each core produces its own [128, 10] shard; the host
concatenates.

Device-side pipeline (all fp32, exact):
  one packed-weights DMA ->
  G.T[16,120] = sum_k expT_k.T @ fc1_w.T_k   (PE, folds h-expansion)
  f1 = relu(G.T.T @ beta2 + b1)              (PE + DVE add/max)
  f2 = relu(fc2_w.T.T @ f1 + b2)             (PE + DVE)
  z  = f2.T @ fc3_w.T + b3                   (PE row-form, no transpose)
  softmax row (DVE max/sum/reciprocal + ACT exp, table preloaded)
  broadcast store via step-0 DMA replication.
"""
import sys
import numpy as np

for _p in ("/opt/trn_rl_repo",):
    if _p not in sys.path:
        sys.path.insert(0, _p)

import concourse.bass as bass  # noqa: E402
import concourse.tile as tile  # noqa: E402
from concourse import bacc, mybir  # noqa: E402
from concourse.bass_utils import run_bass_kernel_spmd  # noqa: E402
from contextlib import ExitStack  # noqa: E402

F32 = mybir.dt.float32
OP = mybir.AluOpType
AF = mybir.ActivationFunctionType
AX = mybir.AxisListType

NCORES = 8
BSHARD = 128

# packed [128, PCOLS] fp32, chunk-local so each chunk is one contiguous DMA:
#  4 blocks of 136 cols at 136k: [fc1_w.T chunk_k (120) | expT chunk_k (16)]
#  544:628  fc2_w.T [0:120]
#  628:638  fc3_w.T [0:84]
#  638      fc1_b col  | 639 fc2_b col | 640 bn2_beta col
#  641:651  fc3_b row [0:1] | 651 one
PCOLS = 652


def _pack_inputs(inputs):
    P = np.zeros((128, PCOLS), dtype=np.float32)
    w1t = np.asarray(inputs["fc1_w"], np.float32).T  # [400, 120]
    expT = np.zeros((400, 16), dtype=np.float32)
    for f in range(400):
        expT[f, f // 25] = 1.0
    for k in range(4):
        P[0:100, 136 * k:136 * k + 120] = w1t[100 * k:100 * k + 100]
        P[0:100, 136 * k + 120:136 * k + 136] = expT[100 * k:100 * k + 100]
    P[0:120, 544:628] = np.asarray(inputs["fc2_w"], np.float32).T
    P[0:84, 628:638] = np.asarray(inputs["fc3_w"], np.float32).T
    P[0:120, 638] = np.asarray(inputs["fc1_b"], np.float32).ravel()
    P[0:84, 639] = np.asarray(inputs["fc2_b"], np.float32).ravel()
    P[0:16, 640] = np.asarray(inputs["bn2_beta"], np.float32).ravel()
    P[0, 641:651] = np.asarray(inputs["fc3_b"], np.float32).ravel()
    P[0, 651] = 1.0
    return {"packed": P}


def _build(nc, tc, ctx):
    pool = ctx.enter_context(tc.tile_pool(name="p", bufs=1))
    psum = ctx.enter_context(tc.tile_pool(name="ps", bufs=1, space="PSUM"))

    pk_d = nc.declare_dram_parameter("packed", [128, PCOLS], F32, isOutput=False)
    out_d = nc.declare_dram_parameter("out", [BSHARD, 10], F32, isOutput=True)

    pk = pool.tile([128, PCOLS], F32)
    # chunked loads: the PE starts on chunk 0 while the rest stream in
    for k in range(4):
        nc.scalar.dma_start(pk[:, 136 * k:136 * k + 136],
                            pk_d[:, 136 * k:136 * k + 136])
    nc.sync.dma_start(pk[:, 544:652], pk_d[:, 544:652])

    # exp-table preload, overlapped with the DMA wait
    warm = pool.tile([1, 1], F32)
    nc.gpsimd.memset(warm[:], 0.0)
    nc.const_aps.aps[(F32, 0.0)] = warm[:]
    nc.scalar.activation(warm[:], warm[:], AF.Exp)

    w1t = lambda k: pk[0:100, 136 * k:136 * k + 120]
    expT = lambda k: pk[0:100, 136 * k + 120:136 * k + 136]
    w2t = pk[0:120, 544:628]
    w3r = pk[0:84, 628:638]
    b1c = pk[0:120, 638:639]
    b2c = pk[0:84, 639:640]
    beta2 = pk[0:16, 640:641]
    b3row = pk[0:1, 641:651]
    ones1 = pk[0:1, 651:652]

    # G.T [16, 120] = sum_k expT_k.T @ w1t_k  (== group-summed fc1_w.T)
    gps = psum.tile([16, 120], F32, name="gps")
    for k in range(4):
        nc.tensor.matmul(gps[:], expT(k), w1t(k), start=(k == 0), stop=(k == 3))
    gt = pool.tile([16, 120], F32)
    nc.vector.tensor_copy(gt[:], gps[:])

    # FC1 + relu (DVE: (psum + b) max 0)
    f1ps = psum.tile([120, 1], F32, name="f1ps")
    nc.tensor.matmul(f1ps[:], gt[:], beta2, start=True, stop=True)
    f1 = pool.tile([120, 1], F32)
    nc.vector.tensor_scalar(f1[:], f1ps[:], b1c, 0.0, OP.add, OP.max)

    # FC2 + relu
    f2ps = psum.tile([84, 1], F32, name="f2ps")
    nc.tensor.matmul(f2ps[:], w2t, f1[:], start=True, stop=True)
    f2 = pool.tile([84, 1], F32)
    nc.vector.tensor_scalar(f2[:], f2ps[:], b2c, 0.0, OP.add, OP.max)

    # FC3 in row form [1, 10]; bias accumulated via K=1 matmul
    zps = psum.tile([1, 10], F32, name="zps")
    nc.tensor.matmul(zps[:], f2[:], w3r, start=True, stop=False)
    nc.tensor.matmul(zps[:], ones1, b3row, start=False, stop=True)

    # softmax on the row
    negmax = pool.tile([1, 1], F32)
    nc.vector.tensor_reduce(negmax[:], zps[:], AX.X, OP.max, negate=True)
    ze = pool.tile([1, 10], F32)
    nc.scalar.activation(ze[:], zps[:], AF.Exp, bias=negmax[:])
    zsum = pool.tile([1, 1], F32)
    nc.vector.tensor_reduce(zsum[:], ze[:], AX.X, OP.add)
    zr = pool.tile([1, 1], F32)
    nc.vector.reciprocal(zr[:], zsum[:])
    prob = pool.tile([1, 10], F32)
    nc.vector.tensor_scalar(prob[:], ze[:], zr[:], None, op0=OP.mult)

    # broadcast-store: step-0 DMA replicates the row to all 128 batch rows
    nc.sync.dma_start(
        out_d[:],
        prob[0:1, :].rearrange("p (a q) -> p a q", a=1).to_broadcast((1, BSHARD, 10)))


def _light_drain_and_barrier(self, tick_clock, wait_clock):
    from concourse.vector_clock import ScopedClock
    drain_inst = self.nc.sync.drain()
    wait_clock.add_sem_waits(drain_inst.ins,
                             ScopedClock({None: tick_clock.global_clock}))
    self.nc.all_engine_barrier()
    popped = self.nc._tile_sem_poison_stack.pop()
    assert popped is self._sem_poison


_COMPILED = None


def _get_compiled():
    global _COMPILED
    if _COMPILED is None:
        nc = bacc.Bacc()
        _orig = tile.TileContext._drain_and_barrier
        tile.TileContext._drain_and_barrier = _light_drain_and_barrier
        try:
            with tile.TileContext(nc) as tc:
                with ExitStack() as ctx:
                    _build(nc, tc, ctx)
        finally:
            tile.TileContext._drain_and_barrier = _orig
        nc.compile()
        _COMPILED = nc
    return _COMPILED


def kernel(**inputs) -> np.ndarray:
    nc = _get_compiled()
    m = _pack_inputs(inputs)
    res = run_bass_kernel_spmd(nc, [dict(m) for _ in range(NCORES)],
                               list(range(NCORES)))
    out = np.concatenate([res.results[c]["out"] for c in range(NCORES)], axis=0)
    batch = int(np.asarray(inputs["x"]).shape[0])
    return out[:batch].astype(np.float32)


# revision 4
# speedup vs baseline: 1.4524x; 1.0702x over previous
"""Trainium2 Bass kernel for nn_CNN_LeNet_83794811945244 (AdderNet LeNet).

Mathematical structure
----------------------
``adder2d`` returns ``-sum |x_patch - w|``, which is **<= 0 for every
possible input** (a negated sum of absolute values).  The reference net
applies ``relu`` directly to each adder output, so both adder stages are
identically zero for ANY input tensors of these shapes:

  * layer1: ``relu(adder2d(x, w1)) == 0`` elementwise; training-mode
    batchnorm of the all-zero tensor is exactly ``beta1`` (the ``0 - mean``
    numerator is exactly 0, so the ``rsqrt(var + eps)`` factor multiplies
    0); maxpool of a constant is that constant.
  * layer2 sees the constant image ``beta1``; again
    ``relu(adder2d(.)) == 0``; bn -> ``beta2``; pool -> ``beta2``.
  * flattened features: ``h[f] = beta2[f // 25]``  (f = (channel, 5, 5)).

Every output row therefore equals
``softmax(fc3_b + fc3_w @ relu(fc2_b + fc2_w @ relu(fc1_b + fc1_w @ h)))``
- input-data independent but *weight*-dependent.  The kernel computes that
row on each NeuronCore from the real ``bn2_beta`` / fc weights (exact fp32
constant-folding of the network; no approximation) and broadcasts it over
its batch shard.

Sharding: pure data parallel over batch (1024 -> 8 x 128) per the hint;
weights replicated.  Each core produces its own [128, 10] shard; the host
concatenates.

Device-side pipeline (all fp32, exact):
  one packed-weights DMA ->
  G.T[16,120] = sum_k expT_k.T @ fc1_w.T_k   (PE, folds h-expansion)
  f1 = relu(G.T.T @ beta2 + b1)              (PE + DVE add/max)
  f2 = relu(fc2_w.T.T @ f1 + b2)             (PE + DVE)
  z  = f2.T @ fc3_w.T + b3                   (PE row-form, no transpose)
  softmax row (DVE max/sum/reciprocal + ACT exp, table preloaded)
  broadcast store via step-0 DMA replication.
"""
import sys
import numpy as np

for _p in ("/opt/trn_rl_repo",):
    if _p not in sys.path:
        sys.path.insert(0, _p)

import concourse.bass as bass  # noqa: E402
import concourse.tile as tile  # noqa: E402
from concourse import bacc, mybir  # noqa: E402
from concourse.bass_utils import run_bass_kernel_spmd  # noqa: E402
from contextlib import ExitStack  # noqa: E402

F32 = mybir.dt.float32
OP = mybir.AluOpType
AF = mybir.ActivationFunctionType
AX = mybir.AxisListType

NCORES = 8
BSHARD = 128

# packed [128, PCOLS] fp32, chunk-local so each chunk is one contiguous DMA:
#  4 blocks of 121 cols at 121k: [fc1_w.T chunk_k (120) | h chunk_k (1)]
#  (h = bn2_beta replicated 25x = the collapsed layer-2 feature column)
#  484:568  fc2_w.T [0:120] | 568:578 fc3_w.T [0:84]
#  578 fc1_b col | 579 fc2_b col | 580:590 fc3_b row [0:1] | 590 one
PCOLS = 591


def _pack_inputs(inputs):
    P = np.zeros((128, PCOLS), dtype=np.float32)
    w1t = np.asarray(inputs["fc1_w"], np.float32).T  # [400, 120]
    h = np.repeat(np.asarray(inputs["bn2_beta"], np.float32).ravel(), 25)
    for k in range(4):
        P[0:100, 121 * k:121 * k + 120] = w1t[100 * k:100 * k + 100]
        P[0:100, 121 * k + 120] = h[100 * k:100 * k + 100]
    P[0:120, 484:568] = np.asarray(inputs["fc2_w"], np.float32).T
    P[0:84, 568:578] = np.asarray(inputs["fc3_w"], np.float32).T
    P[0:120, 578] = np.asarray(inputs["fc1_b"], np.float32).ravel()
    P[0:84, 579] = np.asarray(inputs["fc2_b"], np.float32).ravel()
    P[0, 580:590] = np.asarray(inputs["fc3_b"], np.float32).ravel()
    P[0, 590] = 1.0
    return {"packed": P}


def _build(nc, tc, ctx):
    pool = ctx.enter_context(tc.tile_pool(name="p", bufs=1))
    psum = ctx.enter_context(tc.tile_pool(name="ps", bufs=1, space="PSUM"))

    pk_d = nc.declare_dram_parameter("packed", [128, PCOLS], F32, isOutput=False)
    out_d = nc.declare_dram_parameter("out", [BSHARD, 10], F32, isOutput=True)

    pk = pool.tile([128, PCOLS], F32)
    # chunked loads split across both HWDGE rings; PE starts on chunk 0
    for k in range(4):
        eng = nc.scalar if k % 2 == 0 else nc.sync
        eng.dma_start(pk[:, 121 * k:121 * k + 121],
                      pk_d[:, 121 * k:121 * k + 121])
    nc.sync.dma_start(pk[:, 484:591], pk_d[:, 484:591])

    # exp-table preload, overlapped with the DMA wait
    warm = pool.tile([1, 1], F32)
    nc.gpsimd.memset(warm[:], 0.0)
    nc.const_aps.aps[(F32, 0.0)] = warm[:]
    nc.scalar.activation(warm[:], warm[:], AF.Exp)

    # PE prewarm on memset data: exits the cold p-state while DMAs land
    wz = pool.tile([128, 128], F32)
    nc.gpsimd.memset(wz[:], 0.0)
    wps = psum.tile([128, 128], F32, name="wps")
    for i in range(6):
        nc.tensor.matmul(wps[:], wz[:], wz[:], start=(i == 0), stop=(i == 5))

    w1t = lambda k: pk[0:100, 121 * k:121 * k + 120]
    hc = lambda k: pk[0:100, 121 * k + 120:121 * k + 121]
    w2t = pk[0:120, 484:568]
    w3r = pk[0:84, 568:578]
    b1c = pk[0:120, 578:579]
    b2c = pk[0:84, 579:580]
    b3row = pk[0:1, 580:590]
    ones1 = pk[0:1, 590:591]

    # FC1: f1ps = sum_k fc1_w.T_k.T @ h_k ; relu+bias on DVE
    f1ps = psum.tile([120, 1], F32, name="f1ps")
    for k in range(4):
        nc.tensor.matmul(f1ps[:], w1t(k), hc(k), start=(k == 0), stop=(k == 3))
    f1 = pool.tile([120, 1], F32)
    nc.vector.tensor_scalar(f1[:], f1ps[:], b1c, 0.0, OP.add, OP.max)

    # FC2 + relu
    f2ps = psum.tile([84, 1], F32, name="f2ps")
    nc.tensor.matmul(f2ps[:], w2t, f1[:], start=True, stop=True)
    f2 = pool.tile([84, 1], F32)
    nc.vector.tensor_scalar(f2[:], f2ps[:], b2c, 0.0, OP.add, OP.max)

    # FC3 in row form [1, 10]; bias accumulated via K=1 matmul
    zps = psum.tile([1, 10], F32, name="zps")
    nc.tensor.matmul(zps[:], f2[:], w3r, start=True, stop=False)
    nc.tensor.matmul(zps[:], ones1, b3row, start=False, stop=True)

    # softmax on the row
    negmax = pool.tile([1, 1], F32)
    nc.vector.tensor_reduce(negmax[:], zps[:], AX.X, OP.max, negate=True)
    ze = pool.tile([1, 10], F32)
    nc.scalar.activation(ze[:], zps[:], AF.Exp, bias=negmax[:])
    zsum = pool.tile([1, 1], F32)
    nc.vector.tensor_reduce(zsum[:], ze[:], AX.X, OP.add)
    zr = pool.tile([1, 1], F32)
    nc.vector.reciprocal(zr[:], zsum[:])
    prob = pool.tile([1, 10], F32)
    nc.vector.tensor_scalar(prob[:], ze[:], zr[:], None, op0=OP.mult)

    # broadcast-store: step-0 DMA replicates the row to all 128 batch rows
    nc.sync.dma_start(
        out_d[:],
        prob[0:1, :].rearrange("p (a q) -> p a q", a=1).to_broadcast((1, BSHARD, 10)))


def _light_drain_and_barrier(self, tick_clock, wait_clock):
    from concourse.vector_clock import ScopedClock
    drain_inst = self.nc.sync.drain()
    wait_clock.add_sem_waits(drain_inst.ins,
                             ScopedClock({None: tick_clock.global_clock}))
    self.nc.all_engine_barrier()
    popped = self.nc._tile_sem_poison_stack.pop()
    assert popped is self._sem_poison


_COMPILED = None


def _get_compiled():
    global _COMPILED
    if _COMPILED is None:
        nc = bacc.Bacc()
        _orig = tile.TileContext._drain_and_barrier
        tile.TileContext._drain_and_barrier = _light_drain_and_barrier
        try:
            with tile.TileContext(nc) as tc:
                with ExitStack() as ctx:
                    _build(nc, tc, ctx)
        finally:
            tile.TileContext._drain_and_barrier = _orig
        nc.compile()
        _COMPILED = nc
    return _COMPILED


def kernel(**inputs) -> np.ndarray:
    nc = _get_compiled()
    m = _pack_inputs(inputs)
    res = run_bass_kernel_spmd(nc, [dict(m) for _ in range(NCORES)],
                               list(range(NCORES)))
    out = np.concatenate([res.results[c]["out"] for c in range(NCORES)], axis=0)
    batch = int(np.asarray(inputs["x"]).shape[0])
    return out[:batch].astype(np.float32)
